# revision 19
# baseline (speedup 1.0000x reference)
"""MoE routed dynamics kernel for Trainium2 (8 NeuronCores, expert-parallel).

Problem: for each row b of a [B, D+A] input, route through one of P=8
two-layer MLPs selected by policy_indices[b]:
    h = relu(x @ W1[p] + b1[p]);  y = h @ W2[p] + b2[p]

Sharding: expert-parallel. Core p owns expert p's weights (resident in
SBUF) and processes exactly the rows routed to expert p. The all-to-all
dispatch keyed on policy_indices happens on the host at shard time
(gather rows by expert, pad to a common capacity C), and the inverse
scatter happens at unshard time.

Device kernel (per core), all activations kept feature-on-partition so
no transposes are needed anywhere:
    xT   [DA, C]  (DA=576)         input, transposed on host
    hT   [H, C]   = relu(W1.T @ x + b1), H=1024, via PE matmuls
    outT [D, C]   = W2.T @ h + b2,  D=512
Matmuls run as out[M,N] = lhsT.T @ rhs with lhsT = weight chunks in
their natural [K, M] layout and rhs = activation chunks [K, N<=512].

Matmul dtype is float32r end-to-end (DRAM params, SBUF tiles, and the
relu output): full fp32 operand bits, ~2 PE cycles/row streaming. The
walrus birverifier requires every producer feeding an FP32r matmul to
carry the float32r dtype. Set _MM_DTYPE = "bfloat16" for 1 cycle/row at
bf16 operand precision (host pre-casts inputs).
"""

import math

import numpy as np

_B = 16384
_P = 8
_D = 512
_A = 64
_H = 1024
_DA = _D + _A  # 576
_N_CORES = 8

_MM_DTYPE = "float32r"

_kernel_cache: dict = {}


def _k_chunks(total: int, step: int = 128):
    return [(k0, min(step, total - k0)) for k0 in range(0, total, step)]


def _build_bass(C: int):
    import concourse.bacc as bacc
    import concourse.mybir as mybir
    from concourse.tile import TileContext

    fp32 = mybir.dt.float32
    mmdt = getattr(mybir.dt, _MM_DTYPE)
    act = mybir.ActivationFunctionType

    assert C % 256 == 0, C
    n_chunks = [(n0, min(512, C - n0)) for n0 in range(0, C, 512)]
    k1 = _k_chunks(_DA)   # 5 chunks over DA=576 (last is 64)
    k2 = _k_chunks(_H)    # 8 chunks over H=1024
    mh = _H // 128        # 8 output tiles of layer 1
    md = _D // 128        # 4 output tiles of layer 2

    nc = bacc.Bacc()
    xT = nc.declare_dram_parameter("xT", [_DA, C], mmdt, isOutput=False)
    w1 = nc.declare_dram_parameter("w1", [_DA, _H], mmdt, isOutput=False)
    b1 = nc.declare_dram_parameter("b1", [128, mh], fp32, isOutput=False)
    w2 = nc.declare_dram_parameter("w2", [_H, _D], mmdt, isOutput=False)
    b2 = nc.declare_dram_parameter("b2", [128, md], fp32, isOutput=False)
    outT = nc.declare_dram_parameter("outT", [_D, C], fp32, isOutput=True)

    with TileContext(nc) as tc:
        with (
            tc.tile_pool(name="wpool", bufs=1) as wpool,
            tc.tile_pool(name="xpool", bufs=3) as xpool,
            tc.tile_pool(name="hpool", bufs=2) as hpool,
            tc.tile_pool(name="ypool", bufs=3) as ypool,
            tc.tile_pool(name="ps1", bufs=4, space="PSUM") as ps1,
            tc.tile_pool(name="ps2", bufs=4, space="PSUM") as ps2,
        ):
            def dma_x(n0, nl):
                tiles = []
                for k0, kl in k1:
                    t = xpool.tile([128, nl], mmdt, tag=f"x_{k0}")
                    nc.sync.dma_start(out=t[:kl, :], in_=xT[k0 : k0 + kl, n0 : n0 + nl])
                    tiles.append(t)
                return tiles

            # DMA issue is serial per engine (~0.6us per descriptor set),
            # and ~25 transfers must land before steady state — issuing
            # them all from Sync starves the PE at ~20us. Weights issue
            # from the GpSimd engine (SWDGE queues) in parallel with
            # Sync's activation stream. w1 is split in halves (cols 0:512
            # serve L1 groups m=0..3) so the PE can start sooner.
            w1_sb = []
            for k0, kl in k1:
                wt = wpool.tile([128, _H], mmdt, tag=f"w1_{k0}")
                nc.gpsimd.dma_start(out=wt[:kl, : _H // 2], in_=w1[k0 : k0 + kl, : _H // 2])
                w1_sb.append(wt)
            for i, (k0, kl) in enumerate(k1):
                nc.gpsimd.dma_start(
                    out=w1_sb[i][:kl, _H // 2 :], in_=w1[k0 : k0 + kl, _H // 2 :]
                )
            b1_sb = wpool.tile([128, mh], fp32, tag="b1")
            nc.gpsimd.dma_start(out=b1_sb[:], in_=b1[:, :])
            b2_sb = wpool.tile([128, md], fp32, tag="b2")
            nc.gpsimd.dma_start(out=b2_sb[:], in_=b2[:, :])
            w2_sb = []
            for k0, kl in k2:
                t = wpool.tile([128, _D], mmdt, tag=f"w2_{k0}")
                nc.gpsimd.dma_start(out=t[:kl, :], in_=w2[k0 : k0 + kl, :])
                w2_sb.append(t)
            x_first = dma_x(*n_chunks[0])

            for ci, (n0, nl) in enumerate(n_chunks):
                x_sb = x_first if ci == 0 else dma_x(n0, nl)

                h_sb = []
                for m in range(mh):
                    ps = ps1.tile([128, nl], fp32, tag="ps1")
                    for i, (k0, kl) in enumerate(k1):
                        nc.tensor.matmul(
                            ps[:, :],
                            w1_sb[i][:kl, m * 128 : (m + 1) * 128],
                            x_sb[i][:kl, :],
                            start=(i == 0),
                            stop=(i == len(k1) - 1),
                        )
                    ht = hpool.tile([128, nl], mmdt, tag=f"h_{m}")
                    nc.scalar.activation(ht[:], ps[:], act.Relu, bias=b1_sb[:, m : m + 1])
                    h_sb.append(ht)

                for d in range(md):
                    ps = ps2.tile([128, nl], fp32, tag="ps2")
                    for m in range(mh):
                        nc.tensor.matmul(
                            ps[:, :],
                            w2_sb[m][:, d * 128 : (d + 1) * 128],
                            h_sb[m][:, :],
                            start=(m == 0),
                            stop=(m == mh - 1),
                        )
                    yt = ypool.tile([128, nl], fp32, tag="y")
                    # Bias-add on DVE (idle) instead of ACT (busy with
                    # relu), split in halves so the store of the first half
                    # overlaps the second (shrinks the kernel tail).
                    for h0 in range(0, nl, 256):
                        h1 = min(h0 + 256, nl)
                        nc.vector.tensor_scalar_add(
                            yt[:, h0:h1], ps[:, h0:h1], b2_sb[:, d : d + 1]
                        )
                        nc.sync.dma_start(
                            out=outT[d * 128 : (d + 1) * 128, n0 + h0 : n0 + h1],
                            in_=yt[:, h0:h1],
                        )

    nc.compile()
    return nc


def _get_bass(C: int):
    nc = _kernel_cache.get(C)
    if nc is None:
        nc = _build_bass(C)
        _kernel_cache[C] = nc
    return nc


def _mm_np(a):
    """Cast a float32 array to the numpy dtype matching _MM_DTYPE."""
    if _MM_DTYPE == "bfloat16":
        import ml_dtypes

        return np.ascontiguousarray(a.astype(ml_dtypes.bfloat16))
    return np.ascontiguousarray(a)


def kernel(latents, actions, policy_indices, W1, b1, W2, b2):
    from concourse.bass_utils import run_bass_kernel_spmd

    latents = np.asarray(latents, dtype=np.float32)
    actions = np.asarray(actions, dtype=np.float32)
    pi = np.asarray(policy_indices).astype(np.int64)
    W1 = np.asarray(W1, dtype=np.float32)
    b1 = np.asarray(b1, dtype=np.float32)
    W2 = np.asarray(W2, dtype=np.float32)
    b2 = np.asarray(b2, dtype=np.float32)

    B = latents.shape[0]
    counts = np.bincount(pi, minlength=_P)
    order = np.argsort(pi, kind="stable")
    offs = np.concatenate(([0], np.cumsum(counts)))

    # Common per-core capacity; multiple of 256 so every matmul free dim
    # is >= 256 and chunks are 512 with one optional 256 tail.
    C = max(2304, int(math.ceil(counts.max() / 256)) * 256)

    x = np.empty((B, _DA), dtype=np.float32)
    x[:, :_D] = latents
    x[:, _D:] = actions
    x_sorted = x[order]

    in_maps = []
    for p in range(_P):
        xp = np.zeros((_DA, C), dtype=np.float32)
        xp[:, : counts[p]] = x_sorted[offs[p] : offs[p + 1]].T
        in_maps.append(
            {
                "xT": _mm_np(xp),
                "w1": _mm_np(W1[p]),
                "b1": np.ascontiguousarray(b1[p].reshape(_H // 128, 128).T),
                "w2": _mm_np(W2[p]),
                "b2": np.ascontiguousarray(b2[p].reshape(_D // 128, 128).T),
            }
        )

    nc = _get_bass(C)
    results = run_bass_kernel_spmd(nc, in_maps, list(range(_N_CORES))).results

    out = np.empty((B, _D), dtype=np.float32)
    for p in range(_P):
        yT = results[p]["outT"]
        out[order[offs[p] : offs[p + 1]]] = yT[:, : counts[p]].T
    return out


# revision 20
# speedup vs baseline: 1.0099x; 1.0099x over previous
"""MoE routed dynamics kernel for Trainium2 (8 NeuronCores, expert-parallel).

Problem: for each row b of a [B, D+A] input, route through one of P=8
two-layer MLPs selected by policy_indices[b]:
    h = relu(x @ W1[p] + b1[p]);  y = h @ W2[p] + b2[p]

Sharding: expert-parallel. Core p owns expert p's weights (resident in
SBUF) and processes exactly the rows routed to expert p. The all-to-all
dispatch keyed on policy_indices happens on the host at shard time
(gather rows by expert, pad to a common capacity C), and the inverse
scatter happens at unshard time.

Device kernel (per core), all activations kept feature-on-partition so
no transposes are needed anywhere:
    xT   [DA, C]  (DA=576)         input, transposed on host
    hT   [H, C]   = relu(W1.T @ x + b1), H=1024, via PE matmuls
    outT [D, C]   = W2.T @ h + b2,  D=512
Matmuls run as out[M,N] = lhsT.T @ rhs with lhsT = weight chunks in
their natural [K, M] layout and rhs = activation chunks [K, N<=512].

Matmul dtype is float32r end-to-end (DRAM params, SBUF tiles, and the
relu output): full fp32 operand bits, ~2 PE cycles/row streaming. The
walrus birverifier requires every producer feeding an FP32r matmul to
carry the float32r dtype. Set _MM_DTYPE = "bfloat16" for 1 cycle/row at
bf16 operand precision (host pre-casts inputs).
"""

import math

import numpy as np

_B = 16384
_P = 8
_D = 512
_A = 64
_H = 1024
_DA = _D + _A  # 576
_N_CORES = 8

_MM_DTYPE = "float32r"

_kernel_cache: dict = {}


def _k_chunks(total: int, step: int = 128):
    return [(k0, min(step, total - k0)) for k0 in range(0, total, step)]


def _build_bass(C: int):
    import concourse.bacc as bacc
    import concourse.mybir as mybir
    from concourse.tile import TileContext

    fp32 = mybir.dt.float32
    mmdt = getattr(mybir.dt, _MM_DTYPE)
    act = mybir.ActivationFunctionType

    assert C % 256 == 0, C
    n_chunks = [(n0, min(512, C - n0)) for n0 in range(0, C, 512)]
    k1 = _k_chunks(_DA)   # 5 chunks over DA=576 (last is 64)
    k2 = _k_chunks(_H)    # 8 chunks over H=1024
    mh = _H // 128        # 8 output tiles of layer 1
    md = _D // 128        # 4 output tiles of layer 2

    nc = bacc.Bacc()
    xT = nc.declare_dram_parameter("xT", [_DA, C], mmdt, isOutput=False)
    w1 = nc.declare_dram_parameter("w1", [_DA, _H], mmdt, isOutput=False)
    b1 = nc.declare_dram_parameter("b1", [128, mh], fp32, isOutput=False)
    w2 = nc.declare_dram_parameter("w2", [_H, _D], mmdt, isOutput=False)
    b2 = nc.declare_dram_parameter("b2", [128, md], fp32, isOutput=False)
    outT = nc.declare_dram_parameter("outT", [_D, C], fp32, isOutput=True)

    with TileContext(nc) as tc:
        with (
            tc.tile_pool(name="wpool", bufs=1) as wpool,
            tc.tile_pool(name="xpool", bufs=3) as xpool,
            tc.tile_pool(name="hpool", bufs=2) as hpool,
            tc.tile_pool(name="ypool", bufs=3) as ypool,
            tc.tile_pool(name="ps1", bufs=4, space="PSUM") as ps1,
            tc.tile_pool(name="ps2", bufs=4, space="PSUM") as ps2,
        ):
            def dma_x(n0, nl):
                tiles = []
                for k0, kl in k1:
                    t = xpool.tile([128, nl], mmdt, tag=f"x_{k0}")
                    nc.sync.dma_start(out=t[:kl, :], in_=xT[k0 : k0 + kl, n0 : n0 + nl])
                    tiles.append(t)
                return tiles

            # DMA issue is serial per engine (~0.6us per descriptor set),
            # and ~25 transfers must land before steady state — issuing
            # them all from one engine starves the PE at ~20us. Spread the
            # issue across three streams: x + stores on Sync (HWDGE),
            # w1 on Activation (also HWDGE-capable, idle at startup),
            # w2 + biases on GpSimd (SWDGE). w1 goes in halves (cols
            # 0:512 serve L1 groups m=0..3) so the PE can start sooner.
            w1_sb = []
            for k0, kl in k1:
                wt = wpool.tile([128, _H], mmdt, tag=f"w1_{k0}")
                nc.scalar.dma_start(out=wt[:kl, : _H // 2], in_=w1[k0 : k0 + kl, : _H // 2])
                w1_sb.append(wt)
            for i, (k0, kl) in enumerate(k1):
                nc.scalar.dma_start(
                    out=w1_sb[i][:kl, _H // 2 :], in_=w1[k0 : k0 + kl, _H // 2 :]
                )
            b1_sb = wpool.tile([128, mh], fp32, tag="b1")
            nc.gpsimd.dma_start(out=b1_sb[:], in_=b1[:, :])
            b2_sb = wpool.tile([128, md], fp32, tag="b2")
            nc.gpsimd.dma_start(out=b2_sb[:], in_=b2[:, :])
            w2_sb = []
            for k0, kl in k2:
                t = wpool.tile([128, _D], mmdt, tag=f"w2_{k0}")
                nc.gpsimd.dma_start(out=t[:kl, :], in_=w2[k0 : k0 + kl, :])
                w2_sb.append(t)
            x_first = dma_x(*n_chunks[0])

            for ci, (n0, nl) in enumerate(n_chunks):
                x_sb = x_first if ci == 0 else dma_x(n0, nl)

                h_sb = []
                for m in range(mh):
                    ps = ps1.tile([128, nl], fp32, tag="ps1")
                    for i, (k0, kl) in enumerate(k1):
                        nc.tensor.matmul(
                            ps[:, :],
                            w1_sb[i][:kl, m * 128 : (m + 1) * 128],
                            x_sb[i][:kl, :],
                            start=(i == 0),
                            stop=(i == len(k1) - 1),
                        )
                    ht = hpool.tile([128, nl], mmdt, tag=f"h_{m}")
                    nc.scalar.activation(ht[:], ps[:], act.Relu, bias=b1_sb[:, m : m + 1])
                    h_sb.append(ht)

                for d in range(md):
                    ps = ps2.tile([128, nl], fp32, tag="ps2")
                    for m in range(mh):
                        nc.tensor.matmul(
                            ps[:, :],
                            w2_sb[m][:, d * 128 : (d + 1) * 128],
                            h_sb[m][:, :],
                            start=(m == 0),
                            stop=(m == mh - 1),
                        )
                    yt = ypool.tile([128, nl], fp32, tag="y")
                    # Bias-add on DVE (idle) instead of ACT (busy with
                    # relu), split in halves so the store of the first half
                    # overlaps the second (shrinks the kernel tail).
                    for h0 in range(0, nl, 256):
                        h1 = min(h0 + 256, nl)
                        nc.vector.tensor_scalar_add(
                            yt[:, h0:h1], ps[:, h0:h1], b2_sb[:, d : d + 1]
                        )
                        nc.sync.dma_start(
                            out=outT[d * 128 : (d + 1) * 128, n0 + h0 : n0 + h1],
                            in_=yt[:, h0:h1],
                        )

    nc.compile()
    return nc


def _get_bass(C: int):
    nc = _kernel_cache.get(C)
    if nc is None:
        nc = _build_bass(C)
        _kernel_cache[C] = nc
    return nc


def _mm_np(a):
    """Cast a float32 array to the numpy dtype matching _MM_DTYPE."""
    if _MM_DTYPE == "bfloat16":
        import ml_dtypes

        return np.ascontiguousarray(a.astype(ml_dtypes.bfloat16))
    return np.ascontiguousarray(a)


def kernel(latents, actions, policy_indices, W1, b1, W2, b2):
    from concourse.bass_utils import run_bass_kernel_spmd

    latents = np.asarray(latents, dtype=np.float32)
    actions = np.asarray(actions, dtype=np.float32)
    pi = np.asarray(policy_indices).astype(np.int64)
    W1 = np.asarray(W1, dtype=np.float32)
    b1 = np.asarray(b1, dtype=np.float32)
    W2 = np.asarray(W2, dtype=np.float32)
    b2 = np.asarray(b2, dtype=np.float32)

    B = latents.shape[0]
    counts = np.bincount(pi, minlength=_P)
    order = np.argsort(pi, kind="stable")
    offs = np.concatenate(([0], np.cumsum(counts)))

    # Common per-core capacity; multiple of 256 so every matmul free dim
    # is >= 256 and chunks are 512 with one optional 256 tail.
    C = max(2304, int(math.ceil(counts.max() / 256)) * 256)

    x = np.empty((B, _DA), dtype=np.float32)
    x[:, :_D] = latents
    x[:, _D:] = actions
    x_sorted = x[order]

    in_maps = []
    for p in range(_P):
        xp = np.zeros((_DA, C), dtype=np.float32)
        xp[:, : counts[p]] = x_sorted[offs[p] : offs[p + 1]].T
        in_maps.append(
            {
                "xT": _mm_np(xp),
                "w1": _mm_np(W1[p]),
                "b1": np.ascontiguousarray(b1[p].reshape(_H // 128, 128).T),
                "w2": _mm_np(W2[p]),
                "b2": np.ascontiguousarray(b2[p].reshape(_D // 128, 128).T),
            }
        )

    nc = _get_bass(C)
    results = run_bass_kernel_spmd(nc, in_maps, list(range(_N_CORES))).results

    out = np.empty((B, _D), dtype=np.float32)
    for p in range(_P):
        yT = results[p]["outT"]
        out[order[offs[p] : offs[p + 1]]] = yT[:, : counts[p]].T
    return out


# revision 21
# speedup vs baseline: 1.0595x; 1.0491x over previous
"""MoE routed dynamics kernel for Trainium2 (8 NeuronCores, expert-parallel).

Problem: for each row b of a [B, D+A] input, route through one of P=8
two-layer MLPs selected by policy_indices[b]:
    h = relu(x @ W1[p] + b1[p]);  y = h @ W2[p] + b2[p]

Sharding: expert-parallel. Core p owns expert p's weights (resident in
SBUF) and processes exactly the rows routed to expert p. The all-to-all
dispatch keyed on policy_indices happens on the host at shard time
(gather rows by expert, pad to a common capacity C), and the inverse
scatter happens at unshard time.

Device kernel (per core), all activations kept feature-on-partition so
no transposes are needed anywhere:
    xT   [DA, C]  (DA=576)         input, transposed on host
    hT   [H, C]   = relu(W1.T @ x + b1), H=1024, via PE matmuls
    outT [D, C]   = W2.T @ h + b2,  D=512
Matmuls run as out[M,N] = lhsT.T @ rhs with lhsT = weight chunks in
their natural [K, M] layout and rhs = activation chunks [K, N<=512].

Matmul dtype is float32r end-to-end (DRAM params, SBUF tiles, and the
relu output): full fp32 operand bits, ~2 PE cycles/row streaming. The
walrus birverifier requires every producer feeding an FP32r matmul to
carry the float32r dtype. Set _MM_DTYPE = "bfloat16" for 1 cycle/row at
bf16 operand precision (host pre-casts inputs).
"""

import math

import numpy as np

_B = 16384
_P = 8
_D = 512
_A = 64
_H = 1024
_DA = _D + _A  # 576
_N_CORES = 8

_MM_DTYPE = "float32r"

_kernel_cache: dict = {}


def _k_chunks(total: int, step: int = 128):
    return [(k0, min(step, total - k0)) for k0 in range(0, total, step)]


def _build_bass(C: int):
    import concourse.bacc as bacc
    import concourse.mybir as mybir
    from concourse.tile import TileContext

    fp32 = mybir.dt.float32
    mmdt = getattr(mybir.dt, _MM_DTYPE)
    act = mybir.ActivationFunctionType

    assert C % 256 == 0, C
    n_chunks = [(n0, min(512, C - n0)) for n0 in range(0, C, 512)]
    k1 = _k_chunks(_DA)   # 5 chunks over DA=576 (last is 64)
    k2 = _k_chunks(_H)    # 8 chunks over H=1024
    mh = _H // 128        # 8 output tiles of layer 1
    md = _D // 128        # 4 output tiles of layer 2

    nc = bacc.Bacc()
    xT = nc.declare_dram_parameter("xT", [_DA, C], mmdt, isOutput=False)
    w1 = nc.declare_dram_parameter("w1", [_DA, _H], mmdt, isOutput=False)
    b1 = nc.declare_dram_parameter("b1", [128, mh], fp32, isOutput=False)
    w2 = nc.declare_dram_parameter("w2", [_H, _D], mmdt, isOutput=False)
    b2 = nc.declare_dram_parameter("b2", [128, md], fp32, isOutput=False)
    outT = nc.declare_dram_parameter("outT", [_D, C], fp32, isOutput=True)

    with TileContext(nc) as tc:
        with (
            tc.tile_pool(name="wpool", bufs=1) as wpool,
            tc.tile_pool(name="xpool", bufs=3) as xpool,
            tc.tile_pool(name="hpool", bufs=2) as hpool,
            tc.tile_pool(name="ypool", bufs=3) as ypool,
            tc.tile_pool(name="ps1", bufs=4, space="PSUM") as ps1,
            tc.tile_pool(name="ps2", bufs=4, space="PSUM") as ps2,
        ):
            def dma_x(n0, nl):
                tiles = []
                for k0, kl in k1:
                    t = xpool.tile([128, nl], mmdt, tag=f"x_{k0}")
                    nc.sync.dma_start(out=t[:kl, :], in_=xT[k0 : k0 + kl, n0 : n0 + nl])
                    tiles.append(t)
                return tiles

            # DMA issue on the Sync engine is serial (~0.6us each), so
            # issue order sets how soon the PE can start. Interleave the
            # chunk-0 x tiles with the first-needed halves of w1 (cols
            # 0:512 serve L1 groups m=0..3), then the rest; w2 (needed
            # only ~15us in) goes last.
            x_first = []
            w1_sb = []
            for i, (k0, kl) in enumerate(k1):
                n0, nl = n_chunks[0]
                xt = xpool.tile([128, nl], mmdt, tag=f"x_{k0}")
                nc.sync.dma_start(out=xt[:kl, :], in_=xT[k0 : k0 + kl, n0 : n0 + nl])
                x_first.append(xt)
                wt = wpool.tile([128, _H], mmdt, tag=f"w1_{k0}")
                nc.sync.dma_start(out=wt[:kl, : _H // 2], in_=w1[k0 : k0 + kl, : _H // 2])
                w1_sb.append(wt)
            for i, (k0, kl) in enumerate(k1):
                nc.sync.dma_start(
                    out=w1_sb[i][:kl, _H // 2 :], in_=w1[k0 : k0 + kl, _H // 2 :]
                )
            b1_sb = wpool.tile([128, mh], fp32, tag="b1")
            nc.sync.dma_start(out=b1_sb[:], in_=b1[:, :])
            b2_sb = wpool.tile([128, md], fp32, tag="b2")
            nc.sync.dma_start(out=b2_sb[:], in_=b2[:, :])
            w2_sb = []
            for k0, kl in k2:
                t = wpool.tile([128, _D], mmdt, tag=f"w2_{k0}")
                nc.sync.dma_start(out=t[:kl, :], in_=w2[k0 : k0 + kl, :])
                w2_sb.append(t)

            for ci, (n0, nl) in enumerate(n_chunks):
                x_sb = x_first if ci == 0 else dma_x(n0, nl)

                h_sb = []
                for m in range(mh):
                    ps = ps1.tile([128, nl], fp32, tag="ps1")
                    for i, (k0, kl) in enumerate(k1):
                        nc.tensor.matmul(
                            ps[:, :],
                            w1_sb[i][:kl, m * 128 : (m + 1) * 128],
                            x_sb[i][:kl, :],
                            start=(i == 0),
                            stop=(i == len(k1) - 1),
                        )
                    ht = hpool.tile([128, nl], mmdt, tag=f"h_{m}")
                    nc.scalar.activation(ht[:], ps[:], act.Relu, bias=b1_sb[:, m : m + 1])
                    h_sb.append(ht)

                for d in range(md):
                    ps = ps2.tile([128, nl], fp32, tag="ps2")
                    for m in range(mh):
                        nc.tensor.matmul(
                            ps[:, :],
                            w2_sb[m][:, d * 128 : (d + 1) * 128],
                            h_sb[m][:, :],
                            start=(m == 0),
                            stop=(m == mh - 1),
                        )
                    yt = ypool.tile([128, nl], fp32, tag="y")
                    # Bias-add on DVE (idle) instead of ACT (busy with
                    # relu), split in halves so the store of the first half
                    # overlaps the second (shrinks the kernel tail).
                    for h0 in range(0, nl, 256):
                        h1 = min(h0 + 256, nl)
                        nc.vector.tensor_scalar_add(
                            yt[:, h0:h1], ps[:, h0:h1], b2_sb[:, d : d + 1]
                        )
                        nc.sync.dma_start(
                            out=outT[d * 128 : (d + 1) * 128, n0 + h0 : n0 + h1],
                            in_=yt[:, h0:h1],
                        )

    nc.compile()
    return nc


def _get_bass(C: int):
    nc = _kernel_cache.get(C)
    if nc is None:
        nc = _build_bass(C)
        _kernel_cache[C] = nc
    return nc


def _mm_np(a):
    """Cast a float32 array to the numpy dtype matching _MM_DTYPE."""
    if _MM_DTYPE == "bfloat16":
        import ml_dtypes

        return np.ascontiguousarray(a.astype(ml_dtypes.bfloat16))
    return np.ascontiguousarray(a)


def kernel(latents, actions, policy_indices, W1, b1, W2, b2):
    from concourse.bass_utils import run_bass_kernel_spmd

    latents = np.asarray(latents, dtype=np.float32)
    actions = np.asarray(actions, dtype=np.float32)
    pi = np.asarray(policy_indices).astype(np.int64)
    W1 = np.asarray(W1, dtype=np.float32)
    b1 = np.asarray(b1, dtype=np.float32)
    W2 = np.asarray(W2, dtype=np.float32)
    b2 = np.asarray(b2, dtype=np.float32)

    B = latents.shape[0]
    counts = np.bincount(pi, minlength=_P)
    order = np.argsort(pi, kind="stable")
    offs = np.concatenate(([0], np.cumsum(counts)))

    # Common per-core capacity; multiple of 256 so every matmul free dim
    # is >= 256 and chunks are 512 with one optional 256 tail.
    C = max(2304, int(math.ceil(counts.max() / 256)) * 256)

    x = np.empty((B, _DA), dtype=np.float32)
    x[:, :_D] = latents
    x[:, _D:] = actions
    x_sorted = x[order]

    in_maps = []
    for p in range(_P):
        xp = np.zeros((_DA, C), dtype=np.float32)
        xp[:, : counts[p]] = x_sorted[offs[p] : offs[p + 1]].T
        in_maps.append(
            {
                "xT": _mm_np(xp),
                "w1": _mm_np(W1[p]),
                "b1": np.ascontiguousarray(b1[p].reshape(_H // 128, 128).T),
                "w2": _mm_np(W2[p]),
                "b2": np.ascontiguousarray(b2[p].reshape(_D // 128, 128).T),
            }
        )

    nc = _get_bass(C)
    results = run_bass_kernel_spmd(nc, in_maps, list(range(_N_CORES))).results

    out = np.empty((B, _D), dtype=np.float32)
    for p in range(_P):
        yT = results[p]["outT"]
        out[order[offs[p] : offs[p + 1]]] = yT[:, : counts[p]].T
    return out


# revision 24
# speedup vs baseline: 1.2186x; 1.1501x over previous
"""MoE routed dynamics kernel for Trainium2 (8 NeuronCores, expert-parallel).

Problem: for each row b of a [B, D+A] input, route through one of P=8
two-layer MLPs selected by policy_indices[b]:
    h = relu(x @ W1[p] + b1[p]);  y = h @ W2[p] + b2[p]

Sharding: expert-parallel. Core p owns expert p's weights (resident in
SBUF) and processes exactly the rows routed to expert p. The all-to-all
dispatch keyed on policy_indices happens on the host at shard time
(gather rows by expert, pad to a common capacity C), and the inverse
scatter happens at unshard time.

Device kernel (per core), all activations kept feature-on-partition so
no transposes are needed anywhere:
    xT   [DA, C]  (DA=576)         input, transposed on host
    hT   [H, C]   = relu(W1.T @ x + b1), H=1024, via PE matmuls
    outT [D, C]   = W2.T @ h + b2,  D=512
Matmuls run as out[M,N] = lhsT.T @ rhs with lhsT = weight chunks in
their natural [K, M] layout and rhs = activation chunks [K, N<=512].

Matmul dtype is float32r end-to-end (DRAM params, SBUF tiles, and the
relu output): full fp32 operand bits, ~2 PE cycles/row streaming. The
walrus birverifier requires every producer feeding an FP32r matmul to
carry the float32r dtype. Set _MM_DTYPE = "bfloat16" for 1 cycle/row at
bf16 operand precision (host pre-casts inputs).
"""

import math

import numpy as np

_B = 16384
_P = 8
_D = 512
_A = 64
_H = 1024
_DA = _D + _A   # 576
_DAP = 640      # _DA zero-padded to 5*128: uniform K=128 matmuls (the
                # ragged K=64 tail matmul measurably breaks the PE's
                # LDWEIGHTS pipelining, ~0.3us per L1 group)
_N_CORES = 8

_MM_DTYPE = "float32r"

_kernel_cache: dict = {}


def _k_chunks(total: int, step: int = 128):
    return [(k0, min(step, total - k0)) for k0 in range(0, total, step)]


def _build_bass(C: int):
    import concourse.bacc as bacc
    import concourse.mybir as mybir
    from concourse.tile import TileContext

    fp32 = mybir.dt.float32
    mmdt = getattr(mybir.dt, _MM_DTYPE)
    act = mybir.ActivationFunctionType

    assert C % 256 == 0, C
    n_chunks = [(n0, min(512, C - n0)) for n0 in range(0, C, 512)]
    k1 = _k_chunks(_DAP)  # 5 uniform K=128 chunks over padded DA
    k2 = _k_chunks(_H)    # 8 chunks over H=1024
    mh = _H // 128        # 8 output tiles of layer 1
    md = _D // 128        # 4 output tiles of layer 2

    nc = bacc.Bacc()
    xT = nc.declare_dram_parameter("xT", [_DAP, C], mmdt, isOutput=False)
    w1 = nc.declare_dram_parameter("w1", [_DAP, _H], mmdt, isOutput=False)
    b1 = nc.declare_dram_parameter("b1", [128, mh], fp32, isOutput=False)
    w2 = nc.declare_dram_parameter("w2", [_H, _D], mmdt, isOutput=False)
    b2 = nc.declare_dram_parameter("b2", [128, md], fp32, isOutput=False)
    outT = nc.declare_dram_parameter("outT", [_D, C], fp32, isOutput=True)

    with TileContext(nc) as tc:
        with (
            tc.tile_pool(name="wpool", bufs=1) as wpool,
            tc.tile_pool(name="xpool", bufs=3) as xpool,
            tc.tile_pool(name="hpool", bufs=2) as hpool,
            tc.tile_pool(name="ypool", bufs=3) as ypool,
            tc.tile_pool(name="ps1", bufs=4, space="PSUM") as ps1,
            tc.tile_pool(name="ps2", bufs=4, space="PSUM") as ps2,
        ):
            def dma_x(n0, nl):
                tiles = []
                for k0, kl in k1:
                    t = xpool.tile([128, nl], mmdt, tag=f"x_{k0}")
                    nc.sync.dma_start(out=t[:kl, :], in_=xT[k0 : k0 + kl, n0 : n0 + nl])
                    tiles.append(t)
                return tiles

            # DMA issue on the Sync engine is serial (~0.6us each), so
            # issue order sets how soon the PE can start. Interleave the
            # chunk-0 x tiles with the first-needed halves of w1 (cols
            # 0:512 serve L1 groups m=0..3), then the rest; w2 (needed
            # only ~15us in) goes last.
            x_first = []
            w1_sb = []
            for i, (k0, kl) in enumerate(k1):
                n0, nl = n_chunks[0]
                xt = xpool.tile([128, nl], mmdt, tag=f"x_{k0}")
                nc.sync.dma_start(out=xt[:kl, :], in_=xT[k0 : k0 + kl, n0 : n0 + nl])
                x_first.append(xt)
                wt = wpool.tile([128, _H], mmdt, tag=f"w1_{k0}")
                nc.sync.dma_start(out=wt[:kl, : _H // 2], in_=w1[k0 : k0 + kl, : _H // 2])
                w1_sb.append(wt)
            for i, (k0, kl) in enumerate(k1):
                nc.sync.dma_start(
                    out=w1_sb[i][:kl, _H // 2 :], in_=w1[k0 : k0 + kl, _H // 2 :]
                )
            b1_sb = wpool.tile([128, mh], fp32, tag="b1")
            nc.sync.dma_start(out=b1_sb[:], in_=b1[:, :])
            b2_sb = wpool.tile([128, md], fp32, tag="b2")
            nc.sync.dma_start(out=b2_sb[:], in_=b2[:, :])
            w2_sb = []
            for k0, kl in k2:
                t = wpool.tile([128, _D], mmdt, tag=f"w2_{k0}")
                nc.sync.dma_start(out=t[:kl, :], in_=w2[k0 : k0 + kl, :])
                w2_sb.append(t)

            for ci, (n0, nl) in enumerate(n_chunks):
                x_sb = x_first if ci == 0 else dma_x(n0, nl)

                h_sb = []
                for m in range(mh):
                    ps = ps1.tile([128, nl], fp32, tag="ps1")
                    for i, (k0, kl) in enumerate(k1):
                        nc.tensor.matmul(
                            ps[:, :],
                            w1_sb[i][:kl, m * 128 : (m + 1) * 128],
                            x_sb[i][:kl, :],
                            start=(i == 0),
                            stop=(i == len(k1) - 1),
                        )
                    ht = hpool.tile([128, nl], mmdt, tag=f"h_{m}")
                    nc.scalar.activation(ht[:], ps[:], act.Relu, bias=b1_sb[:, m : m + 1])
                    h_sb.append(ht)

                for d in range(md):
                    ps = ps2.tile([128, nl], fp32, tag="ps2")
                    for m in range(mh):
                        nc.tensor.matmul(
                            ps[:, :],
                            w2_sb[m][:, d * 128 : (d + 1) * 128],
                            h_sb[m][:, :],
                            start=(m == 0),
                            stop=(m == mh - 1),
                        )
                    yt = ypool.tile([128, nl], fp32, tag="y")
                    # Bias-add on DVE (idle) instead of ACT (busy with
                    # relu), split in halves so the store of the first half
                    # overlaps the second (shrinks the kernel tail).
                    for h0 in range(0, nl, 256):
                        h1 = min(h0 + 256, nl)
                        nc.vector.tensor_scalar_add(
                            yt[:, h0:h1], ps[:, h0:h1], b2_sb[:, d : d + 1]
                        )
                        nc.sync.dma_start(
                            out=outT[d * 128 : (d + 1) * 128, n0 + h0 : n0 + h1],
                            in_=yt[:, h0:h1],
                        )

    nc.compile()
    return nc


def _get_bass(C: int):
    nc = _kernel_cache.get(C)
    if nc is None:
        nc = _build_bass(C)
        _kernel_cache[C] = nc
    return nc


def _mm_np(a):
    """Cast a float32 array to the numpy dtype matching _MM_DTYPE."""
    if _MM_DTYPE == "bfloat16":
        import ml_dtypes

        return np.ascontiguousarray(a.astype(ml_dtypes.bfloat16))
    return np.ascontiguousarray(a)


def _prepare_in_maps(latents, actions, policy_indices, W1, b1, W2, b2):
    """Expert-parallel dispatch: returns (in_maps, C, order, offs, counts)."""
    latents = np.asarray(latents, dtype=np.float32)
    actions = np.asarray(actions, dtype=np.float32)
    pi = np.asarray(policy_indices).astype(np.int64)
    W1 = np.asarray(W1, dtype=np.float32)
    b1 = np.asarray(b1, dtype=np.float32)
    W2 = np.asarray(W2, dtype=np.float32)
    b2 = np.asarray(b2, dtype=np.float32)

    B = latents.shape[0]
    counts = np.bincount(pi, minlength=_P)
    order = np.argsort(pi, kind="stable")
    offs = np.concatenate(([0], np.cumsum(counts)))

    # Common per-core capacity; multiple of 256 so every matmul free dim
    # is >= 256 and chunks are 512 with one optional 256 tail.
    C = max(2304, int(math.ceil(counts.max() / 256)) * 256)

    x = np.empty((B, _DA), dtype=np.float32)
    x[:, :_D] = latents
    x[:, _D:] = actions
    x_sorted = x[order]

    in_maps = []
    for p in range(_P):
        xp = np.zeros((_DAP, C), dtype=np.float32)
        xp[:_DA, : counts[p]] = x_sorted[offs[p] : offs[p + 1]].T
        w1p = np.zeros((_DAP, _H), dtype=np.float32)
        w1p[:_DA] = W1[p]
        in_maps.append(
            {
                "xT": _mm_np(xp),
                "w1": _mm_np(w1p),
                "b1": np.ascontiguousarray(b1[p].reshape(_H // 128, 128).T),
                "w2": _mm_np(W2[p]),
                "b2": np.ascontiguousarray(b2[p].reshape(_D // 128, 128).T),
            }
        )
    return in_maps, C, order, offs, counts


def kernel(latents, actions, policy_indices, W1, b1, W2, b2):
    from concourse.bass_utils import run_bass_kernel_spmd

    in_maps, C, order, offs, counts = _prepare_in_maps(
        latents, actions, policy_indices, W1, b1, W2, b2
    )
    nc = _get_bass(C)
    results = run_bass_kernel_spmd(nc, in_maps, list(range(_N_CORES))).results

    B = np.asarray(latents).shape[0]
    out = np.empty((B, _D), dtype=np.float32)
    for p in range(_P):
        yT = results[p]["outT"]
        out[order[offs[p] : offs[p + 1]]] = yT[:, : counts[p]].T
    return out
